# revision 18
# baseline (speedup 1.0000x reference)
"""Trainium2 Bass kernel for nn_CausalSelfAttention_74268574482879.

The reference module's attention scores are overwritten by the causal mask
(q/k are discarded), so softmax weights are uniform over positions <= t:
    y = cummean_T(x) @ W_v @ W_p,   W_v = w_attn[:, 1024:1536]

Host-side prep (weight folding + shard slicing):
  Wc = W_v @ W_p folded once on host (weight-only preprocessing), bf16.
  x shards shipped bf16 feature-major with the cross-shard halo in col 0
  and the 1/(t+1) eviction scales as fp32-viewed-as-bf16-pair columns.

Per-core dataflow (bf16 end-to-end, fp32 accumulation):
  in   : sync ring  [xt0, xt1, wc1, wc2], scalar ring [wc0, xt2, xt3, wc3]
         (wc0 first so PE round 0 is never weight-gated; x slices land in
         scan order; ~128KB per transfer, both rings drain concurrently)
  scan : 16 chained 128-col DVE sub-scans (initial = prev col / halo col)
  mm   : psY[tt] += At[i][:, tt]^T @ Wc[i]  (PE, rounds i=0..3, tt inner;
         each mm chases its sub-scan + its wc tile at 128-col granularity)
  evict: per tt, 3 engine-parallel thirds (ACT/DVE/GPSIMD) scaling by the
         per-partition 1/(t+1) fp32 scale, bf16 out
  out  : 4 per-tt y transfers alternating sync/scalar rings
"""

import numpy as np
import ml_dtypes

import concourse.bass as bass
import concourse.bacc as bacc
import concourse.mybir as mybir
import concourse.tile as tile
from concourse import bass_utils

N_CORES = 8
B, T, C = 2, 2048, 512
CHUNK = 512               # tokens per core
P = 128
NT = CHUNK // P           # 4 token-tiles
NI = C // P               # 4 feature-slices
XW = 516                  # halo col + 512 x cols + pad + 2 scale cols (f32 as bf16 pair)
F32 = mybir.dt.float32
BF16 = mybir.dt.bfloat16
BF16_NP = ml_dtypes.bfloat16

N_WARMUP = [30]           # junk N=128 matmuls at t=0 (PE p-state warm-up)
GAP_JUNK = [(0, 0, 0, 0)]  # junk mms after r0/r1/r2/r3a (PE already full-speed)
N_DEAD = [0]              # dead ACT ops pacing the wc ring behind x
GPS_EVICT = [False]
TRACE = [False]
LAST_RESULT = [None]
_STATE = {}


def _build_nc(cfg):
    n_warmup, gap_junk, n_dead, gps_evict = cfg
    nc = bacc.Bacc(
        "TRN2", target_bir_lowering=False, debug=False, num_devices=N_CORES
    )

    xt_d = nc.dram_tensor("xt", (C, XW), BF16, kind="ExternalInput")
    # wc host-shuffled to (P, NI*C): wc[p, i*C + n] = Wc[i*P + p, n]
    wc_d = nc.dram_tensor("wc", (P, NI * C), BF16, kind="ExternalInput")
    y_d = nc.dram_tensor("y", (CHUNK, C), BF16, kind="ExternalOutput")

    xt_ap, wc_ap, y_ap = xt_d.ap(), wc_d.ap(), y_d.ap()

    with tile.TileContext(nc) as tc:
        with (
            tc.tile_pool(name="io", bufs=1) as io,
            tc.tile_pool(name="ps", bufs=5, space="PSUM") as ps,
        ):
            # ---- PE p-state warm-up junk matmuls ----
            junk = io.tile([P, P], BF16, name="junk")
            nc.vector.memset(junk[:], 1.0)
            psj = ps.tile([P, C], F32, name="psj", tag="junk", bufs=1)

            def junk_mms(n):
                for k in range(n):
                    nc.tensor.matmul(
                        psj[:, (k % NT) * P : (k % NT + 1) * P],
                        junk[:],
                        junk[:],
                        start=True,
                        stop=True,
                        skip_group_check=True,
                    )

            junk_mms(n_warmup)

            if n_dead:
                dead = io.tile([P, P], BF16, name="dead")
                for _ in range(n_dead):
                    nc.scalar.mul(dead[:], junk[:], 1.0)

            # ---- input DMAs (authoring order = per-engine issue order) ----
            xt_sb = [io.tile([P, XW], BF16, name=f"xt{i}") for i in range(NI)]
            wc_sb = [io.tile([P, C], BF16, name=f"wc{i}") for i in range(NI)]

            def dma_xt(eng, i):
                eng.dma_start(xt_sb[i][:], xt_ap[i * P : (i + 1) * P, :])

            def dma_wc(eng, i):
                eng.dma_start(wc_sb[i][:], wc_ap[:, i * C : (i + 1) * C])

            # x interleaved across both HW rings (scan inputs, earliest);
            # xt0 ships as two half-column transfers so scan 0 starts as
            # soon as the first 66KB lands; wc rides the gpsimd SW ring —
            # its ~1us/transfer desc-gen cost self-paces it behind x
            HA = 1 + CHUNK // 2  # halo col + first 256 x cols
            nc.sync.dma_start(xt_sb[0][:, 0:HA], xt_ap[0:P, 0:HA])
            dma_xt(nc.scalar, 1)
            nc.sync.dma_start(xt_sb[0][:, HA:XW], xt_ap[0:P, HA:XW])
            dma_xt(nc.scalar, 3)
            dma_xt(nc.sync, 2)
            for i in range(NI):
                dma_wc(nc.gpsimd, i)

            # ---- prefix scans on DVE ----
            # slice 0 in halves (chases the split xt0 DMA), slices 1-2 full,
            # slice 3 as 384+128 so only the tt3 matmul waits for the tail
            # At[i][:, t] = halo_i + cumsum_{s <= t} x^T_i[:, s]
            SPLITS = {
                0: [CHUNK // 2, CHUNK // 2],
                1: [CHUNK],
                2: [CHUNK],
                3: [3 * CHUNK // 4, CHUNK // 4],
            }
            At = []
            for i in range(NI):
                a = io.tile([P, CHUNK], BF16, name=f"At{i}")
                s = 0
                for w in SPLITS[i]:
                    e = s + w
                    init = xt_sb[i][:, 0:1] if s == 0 else a[:, s - 1 : s]
                    nc.vector.tensor_tensor_scan(
                        a[:, s:e],
                        xt_sb[i][:, 1 + s : 1 + e],
                        xt_sb[i][:, 1 + s : 1 + e],
                        init,
                        mybir.AluOpType.add,
                        mybir.AluOpType.bypass,
                    )
                    s = e
                At.append(a)

            # ---- Y = A @ Wc, accumulated over feature slices i ----
            psY = [
                ps.tile([P, C], F32, name=f"psY{tt}", tag="y", bufs=4)
                for tt in range(NT)
            ]
            for i in range(NI):
                for tt in range(NT):
                    nc.tensor.matmul(
                        psY[tt][:],
                        At[i][:, tt * P : (tt + 1) * P],
                        wc_sb[i][:],
                        start=(i == 0),
                        stop=(i == NI - 1),
                    )
                    if i == NI - 1 and tt == 1 and gap_junk[3]:
                        junk_mms(gap_junk[3])
                if i < NI - 1 and gap_junk[i]:
                    junk_mms(gap_junk[i])

            # ---- evict with fused 1/(t+1) scale ----
            # tt0 on ACT (its wait overlaps DVE's last scan half), rest on
            # DVE full-tile (tensor_scalar f32->bf16 measured ~400ns)
            ysb = [io.tile([P, C], BF16, name=f"y{tt}") for tt in range(NT)]
            for tt in range(NT):
                scale = xt_sb[tt][:, CHUNK + 2 : CHUNK + 4].bitcast(F32)
                if tt % 2 == 0:
                    nc.scalar.mul(ysb[tt][:], psY[tt][:], scale)
                else:
                    nc.vector.tensor_scalar_mul(ysb[tt][:], psY[tt][:], scale)

            y_r = y_ap.rearrange("(k p) n -> k p n", p=P)
            for tt in range(NT):
                eng = nc.sync if tt % 2 == 0 else nc.scalar
                eng.dma_start(y_r[tt], ysb[tt][:])

    nc.compile()
    return nc


def _get_nc():
    key = (N_WARMUP[0], GAP_JUNK[0], N_DEAD[0], GPS_EVICT[0])
    if key not in _STATE:
        _STATE[key] = _build_nc(key)
    return _STATE[key]


def _prepare_in_maps(x, w_attn, w_proj):
    x = np.asarray(x, dtype=np.float32)
    w_attn = np.asarray(w_attn, dtype=np.float32)
    w_proj = np.asarray(w_proj, dtype=np.float32)
    wc_full = (w_attn[:, 2 * C : 3 * C] @ w_proj).astype(BF16_NP)
    # shuffle to (P, NI*C): wc[p, i*C + n] = Wc[i*P + p, n]
    wc = np.ascontiguousarray(
        wc_full.reshape(NI, P, C).transpose(1, 0, 2).reshape(P, NI * C)
    )

    in_maps = []
    for core in range(N_CORES):
        b, tc = divmod(core, T // CHUNK)
        goff = tc * CHUNK
        halo = (
            x[b, :goff, :].sum(axis=0, dtype=np.float32)
            if goff
            else np.zeros(C, np.float32)
        )
        scale = (1.0 / (goff + np.arange(1, CHUNK + 1))).astype(np.float32)
        xt = np.zeros((C, XW), dtype=BF16_NP)
        xt[:, 0] = halo.astype(BF16_NP)
        xt[:, 1 : CHUNK + 1] = x[b, goff : goff + CHUNK, :].T.astype(BF16_NP)
        # per-slice eviction scales: slice i rows carry the fp32 scales for
        # tokens i*P..(i+1)*P as raw bytes viewed as a bf16 pair
        sc_f32 = scale.reshape(NT, P)  # row i = scales for psY[i] partitions
        for i in range(NT):
            xt[i * P : (i + 1) * P, CHUNK + 2 : CHUNK + 4] = (
                np.ascontiguousarray(sc_f32[i]).view(BF16_NP).reshape(P, 2)
            )
        in_maps.append({"xt": np.ascontiguousarray(xt), "wc": wc})
    return in_maps


def kernel(x, w_attn, w_proj):
    nc = _get_nc()
    in_maps = _prepare_in_maps(x, w_attn, w_proj)
    res = bass_utils.run_bass_kernel_spmd(
        nc, in_maps, core_ids=list(range(N_CORES)), trace=TRACE[0]
    )
    LAST_RESULT[0] = res
    y = np.empty((B, T, C), np.float32)
    for core in range(N_CORES):
        b, tc = divmod(core, T // CHUNK)
        y[b, tc * CHUNK : (tc + 1) * CHUNK, :] = res.results[core][
            "y"
        ].astype(np.float32)
    return y


# revision 22
# speedup vs baseline: 1.0426x; 1.0426x over previous
"""Trainium2 Bass kernel for nn_CausalSelfAttention_74268574482879.

The reference module's attention scores are overwritten by the causal mask
(q/k are discarded), so softmax weights are uniform over positions <= t:
    y = cummean_T(x) @ W_v @ W_p,   W_v = w_attn[:, 1024:1536]

Host-side prep (weight folding + shard slicing):
  Wc = W_v @ W_p folded once on host (weight-only preprocessing), bf16.
  x shards shipped bf16 feature-major with the cross-shard halo in col 0
  and the 1/(t+1) eviction scales as fp32-viewed-as-bf16-pair columns.

Per-core dataflow (bf16 end-to-end, fp32 accumulation):
  in   : sync ring  [xt0, xt1, wc1, wc2], scalar ring [wc0, xt2, xt3, wc3]
         (wc0 first so PE round 0 is never weight-gated; x slices land in
         scan order; ~128KB per transfer, both rings drain concurrently)
  scan : 16 chained 128-col DVE sub-scans (initial = prev col / halo col)
  mm   : psY[tt] += At[i][:, tt]^T @ Wc[i]  (PE, rounds i=0..3, tt inner;
         each mm chases its sub-scan + its wc tile at 128-col granularity)
  evict: per tt, 3 engine-parallel thirds (ACT/DVE/GPSIMD) scaling by the
         per-partition 1/(t+1) fp32 scale, bf16 out
  out  : 4 per-tt y transfers alternating sync/scalar rings
"""

import numpy as np
import ml_dtypes

import concourse.bass as bass
import concourse.bacc as bacc
import concourse.mybir as mybir
import concourse.tile as tile
from concourse import bass_utils

N_CORES = 8
B, T, C = 2, 2048, 512
CHUNK = 512               # tokens per core
P = 128
NT = CHUNK // P           # 4 token-tiles
NI = C // P               # 4 feature-slices
XW = 516                  # halo col + 512 x cols + pad + 2 scale cols (f32 as bf16 pair)
F32 = mybir.dt.float32
BF16 = mybir.dt.bfloat16
BF16_NP = ml_dtypes.bfloat16

N_WARMUP = [30]           # junk N=128 matmuls at t=0 (PE p-state warm-up)
GAP_JUNK = [(0, 0, 0, 0)]  # junk mms after r0/r1/r2/r3a (PE already full-speed)
N_DEAD = [0]              # dead ACT ops pacing the wc ring behind x
GPS_EVICT = [False]
TRACE = [False]
LAST_RESULT = [None]
_STATE = {}


def _build_nc(cfg):
    n_warmup, gap_junk, n_dead, gps_evict = cfg
    nc = bacc.Bacc(
        "TRN2", target_bir_lowering=False, debug=False, num_devices=N_CORES
    )

    xt_d = nc.dram_tensor("xt", (C, XW), BF16, kind="ExternalInput")
    # wc host-shuffled to (P, NI*C): wc[p, i*C + n] = Wc[i*P + p, n]
    wc_d = nc.dram_tensor("wc", (P, NI * C), BF16, kind="ExternalInput")
    y_d = nc.dram_tensor("y", (CHUNK, C), BF16, kind="ExternalOutput")

    xt_ap, wc_ap, y_ap = xt_d.ap(), wc_d.ap(), y_d.ap()

    with tile.TileContext(nc) as tc:
        with (
            tc.tile_pool(name="io", bufs=1) as io,
            tc.tile_pool(name="ps", bufs=5, space="PSUM") as ps,
        ):
            # ---- PE p-state warm-up junk matmuls ----
            junk = io.tile([P, P], BF16, name="junk")
            nc.vector.memset(junk[:], 1.0)
            psj = ps.tile([P, C], F32, name="psj", tag="junk", bufs=1)

            def junk_mms(n):
                for k in range(n):
                    nc.tensor.matmul(
                        psj[:, (k % NT) * P : (k % NT + 1) * P],
                        junk[:],
                        junk[:],
                        start=True,
                        stop=True,
                        skip_group_check=True,
                    )

            junk_mms(n_warmup)

            if n_dead:
                dead = io.tile([P, P], BF16, name="dead")
                for _ in range(n_dead):
                    nc.scalar.mul(dead[:], junk[:], 1.0)

            # ---- input DMAs (authoring order = per-engine issue order) ----
            xt_sb = [io.tile([P, XW], BF16, name=f"xt{i}") for i in range(NI)]

            def dma_xt(eng, i):
                eng.dma_start(xt_sb[i][:], xt_ap[i * P : (i + 1) * P, :])

            # x interleaved across both HW rings (scan inputs, earliest);
            # wc on the gpsimd SW ring — its ~1us/transfer desc-gen cost
            # self-paces it behind x. SW-ring completion sems ladder ~1.1us
            # apart, so wc2+wc3 merge into one transfer (round 3 would
            # otherwise be gated by wc3's 4th-in-ladder receipt).
            dma_xt(nc.sync, 0)
            dma_xt(nc.scalar, 1)
            dma_xt(nc.sync, 2)
            dma_xt(nc.scalar, 3)
            wc0 = io.tile([P, C], BF16, name="wc0")
            wc1 = io.tile([P, C], BF16, name="wc1")
            wc23 = io.tile([P, 2 * C], BF16, name="wc23")
            nc.gpsimd.dma_start(wc0[:], wc_ap[:, 0:C])
            nc.gpsimd.dma_start(wc1[:], wc_ap[:, C : 2 * C])
            nc.gpsimd.dma_start(wc23[:], wc_ap[:, 2 * C : 4 * C])
            wc_use = [wc0[:], wc1[:], wc23[:, 0:C], wc23[:, C : 2 * C]]

            # ---- prefix scans on DVE ----
            # slice 0 in halves (chases the split xt0 DMA), slices 1-2 full,
            # slice 3 as 384+128 so only the tt3 matmul waits for the tail
            # At[i][:, t] = halo_i + cumsum_{s <= t} x^T_i[:, s]
            SPLITS = {
                0: [CHUNK // 2, CHUNK // 2],
                1: [CHUNK],
                2: [CHUNK],
                3: [3 * CHUNK // 4, CHUNK // 4],
            }
            At = []
            for i in range(NI):
                a = io.tile([P, CHUNK], BF16, name=f"At{i}")
                s = 0
                for w in SPLITS[i]:
                    e = s + w
                    init = xt_sb[i][:, 0:1] if s == 0 else a[:, s - 1 : s]
                    nc.vector.tensor_tensor_scan(
                        a[:, s:e],
                        xt_sb[i][:, 1 + s : 1 + e],
                        xt_sb[i][:, 1 + s : 1 + e],
                        init,
                        mybir.AluOpType.add,
                        mybir.AluOpType.bypass,
                    )
                    s = e
                At.append(a)

            # ---- Y = A @ Wc, accumulated over feature slices i ----
            psY = [
                ps.tile([P, C], F32, name=f"psY{tt}", tag="y", bufs=4)
                for tt in range(NT)
            ]
            for i in range(NI):
                for tt in range(NT):
                    nc.tensor.matmul(
                        psY[tt][:],
                        At[i][:, tt * P : (tt + 1) * P],
                        wc_use[i],
                        start=(i == 0),
                        stop=(i == NI - 1),
                    )
                    if i == NI - 1 and tt == 1 and gap_junk[3]:
                        junk_mms(gap_junk[3])
                if i < NI - 1 and gap_junk[i]:
                    junk_mms(gap_junk[i])

            # ---- evict with fused 1/(t+1) scale ----
            # tt0 on ACT (its wait overlaps DVE's last scan half), rest on
            # DVE full-tile (tensor_scalar f32->bf16 measured ~400ns)
            ysb = [io.tile([P, C], BF16, name=f"y{tt}") for tt in range(NT)]
            for tt in range(NT):
                scale = xt_sb[tt][:, CHUNK + 2 : CHUNK + 4].bitcast(F32)
                if tt % 2 == 0:
                    nc.scalar.mul(ysb[tt][:], psY[tt][:], scale)
                else:
                    nc.vector.tensor_scalar_mul(ysb[tt][:], psY[tt][:], scale)

            y_r = y_ap.rearrange("(k p) n -> k p n", p=P)
            for tt in range(NT):
                eng = nc.sync if tt % 2 == 0 else nc.scalar
                eng.dma_start(y_r[tt], ysb[tt][:])

    nc.compile()
    return nc


def _get_nc():
    key = (N_WARMUP[0], GAP_JUNK[0], N_DEAD[0], GPS_EVICT[0])
    if key not in _STATE:
        _STATE[key] = _build_nc(key)
    return _STATE[key]


def _prepare_in_maps(x, w_attn, w_proj):
    x = np.asarray(x, dtype=np.float32)
    w_attn = np.asarray(w_attn, dtype=np.float32)
    w_proj = np.asarray(w_proj, dtype=np.float32)
    wc_full = (w_attn[:, 2 * C : 3 * C] @ w_proj).astype(BF16_NP)
    # shuffle to (P, NI*C): wc[p, i*C + n] = Wc[i*P + p, n]
    wc = np.ascontiguousarray(
        wc_full.reshape(NI, P, C).transpose(1, 0, 2).reshape(P, NI * C)
    )

    in_maps = []
    for core in range(N_CORES):
        b, tc = divmod(core, T // CHUNK)
        goff = tc * CHUNK
        halo = (
            x[b, :goff, :].sum(axis=0, dtype=np.float32)
            if goff
            else np.zeros(C, np.float32)
        )
        scale = (1.0 / (goff + np.arange(1, CHUNK + 1))).astype(np.float32)
        xt = np.zeros((C, XW), dtype=BF16_NP)
        xt[:, 0] = halo.astype(BF16_NP)
        xt[:, 1 : CHUNK + 1] = x[b, goff : goff + CHUNK, :].T.astype(BF16_NP)
        # per-slice eviction scales: slice i rows carry the fp32 scales for
        # tokens i*P..(i+1)*P as raw bytes viewed as a bf16 pair
        sc_f32 = scale.reshape(NT, P)  # row i = scales for psY[i] partitions
        for i in range(NT):
            xt[i * P : (i + 1) * P, CHUNK + 2 : CHUNK + 4] = (
                np.ascontiguousarray(sc_f32[i]).view(BF16_NP).reshape(P, 2)
            )
        in_maps.append({"xt": np.ascontiguousarray(xt), "wc": wc})
    return in_maps


def kernel(x, w_attn, w_proj):
    nc = _get_nc()
    in_maps = _prepare_in_maps(x, w_attn, w_proj)
    res = bass_utils.run_bass_kernel_spmd(
        nc, in_maps, core_ids=list(range(N_CORES)), trace=TRACE[0]
    )
    LAST_RESULT[0] = res
    y = np.empty((B, T, C), np.float32)
    for core in range(N_CORES):
        b, tc = divmod(core, T // CHUNK)
        y[b, tc * CHUNK : (tc + 1) * CHUNK, :] = res.results[core][
            "y"
        ].astype(np.float32)
    return y
